# revision 1
# baseline (speedup 1.0000x reference)
"""MultiHeadAttention forward on 8 Trainium2 NeuronCores (Bass/Tile).

Problem: B=2, N=2048, C=1024, H=16, D=64, fp32.
  q/k/v = x @ W* + b*; scores = q k^T / sqrt(D); w = softmax(scores);
  out = (w v) @ Wo + bo.

Sharding: tensor-parallel over (batch, head-group). Core c handles batch
b = c//4 and heads 4*(c%4)..4*(c%4)+3 (channel slice of 256). Each core
computes its own Q/K/V projections (no duplication), attention for its 4
heads, and a PARTIAL output projection out_part = att @ Wo[ch, :]. The
host sums the 4 partials per batch during unshard (row-parallel linear);
bo/4 is added on every core so the partial sum reproduces +bo exactly.

HW-calibrated notes (measured on this machine): every f32r matmul pays a
~220ns fixed cost (self-loading 4-byte weight load); 64-partition
contraction costs ~+120ns more; back-to-back matmuls with the SAME
stationary operand save ~75ns. Hence:
  - K is stored zero-PADDED per head (kt_pad[h] = [K_h;0] / [0;K_h]) so
    score matmuls contract over all 128 partitions (the zero rows kill
    the other head's Q rows that share the moving operand).
  - All matmul loops are ordered so consecutive matmuls share their
    stationary operand (scores/AV pairs across two query blocks; Q/K/C
    chain 4 query chunks per weight slice).
  - Broadcasts (softmax 1/denom row, bv bias) use PE ones-matmuls
    (GPSIMD partition_broadcast gives wrong results on HW for source
    rows at partition offsets > 0).

Per-core pipeline (all matmuls f32r, fp32 PSUM accumulation):
  A: QT[256,2048] = Wq_ch^T @ xT (+bq, DVE), K likewise into kt_pad;
     V[2048,256] computed directly in keys-major layout (xT stationary),
     +bv via a PE-broadcast bias tile. V stored as per-head-pair
     panels [V_even |ones| junk | V_odd] so each AV matmul yields values
     on the head's native partitions plus a denominator row (ones col).
  B: per head h, per query-block pair: for each key tile: scoresT
     [128,1024] (both blocks, shared stationary); et = exp(0.125*s)
     (ACT); av[128,512] x2 += panel^T et over 16 key tiles. Normalize:
     DVE reciprocal of denom row + PE broadcast + DVE multiply.
  C: out_part^T [1024,2048] = Wo_ch^T @ attT (+bo/4, DVE/ACT), DMA out.

Host pre-tiles every input so each DMA is one large contiguous transfer
(DMA-issue descriptor cost dominated the original kernel); all loads go
on the otherwise-idle GPSIMD queue so rep r+1's transfers hide under rep
r's phase B.
"""
import os
import numpy as np

B, N, C, H, D = 2, 2048, 1024, 16, 64
NCORES = 8
HPC = 4              # heads per core
CHC = HPC * D        # channels per core = 256
KT_TILES = 8         # contraction tiles over C
RT_TILES = 16        # key tiles over N

# v_sb per key-tile column layout: two pairs of 192 cols:
#   [V_h0(0:64) | ones(64) | junk(65:128) | V_h1(128:192)]
#   [V_h2(192:256) | ones(256) | junk(257:320) | V_h3(320:384)]
# (the junk region maps to av output rows nobody reads)
V_COLS = 384
V_STORE = (0, 128, 192, 320)   # where head h's V values are stored
# lhsT slices for the AV matmul: even heads [V|1] (65 cols, denom row 64);
# odd heads [1|junk|V] (128 cols, denom row 0, values rows 64..127)
V_LHS = ((0, 65), (64, 128), (192, 65), (256, 128))

_CACHE = {}


def _build_nc(nrep: int = 1, kdtype: str = "f32r", small_out: bool = False,
              phases: str = "ABC", vd_bf16: bool = False):
    import concourse.bacc as bacc
    import concourse.mybir as mybir
    import concourse.tile as tile
    from concourse.bass import ts, ds

    f32 = mybir.dt.float32
    fr = mybir.dt.float32r if kdtype == "f32r" else mybir.dt.float32

    nc = bacc.Bacc("TRN2", target_bir_lowering=False, debug=False)

    # ---- I/O (host pre-tiled; every load is one contiguous DMA) ----
    xt_d = nc.dram_tensor("xt", [128, KT_TILES * N], fr, kind="ExternalInput")
    wq_d = nc.dram_tensor("wq", [128, KT_TILES * CHC], fr, kind="ExternalInput")
    wk_d = nc.dram_tensor("wk", [128, KT_TILES * CHC], fr, kind="ExternalInput")
    wv_d = nc.dram_tensor("wv", [128, KT_TILES * CHC], fr, kind="ExternalInput")
    wo_d = nc.dram_tensor("wo", [128, 2 * C], fr, kind="ExternalInput")
    bqk_d = nc.dram_tensor("bqk", [128, 4], f32, kind="ExternalInput")
    bv_d = nc.dram_tensor("bv", [1, CHC], fr, kind="ExternalInput")
    bo4_d = nc.dram_tensor("bo4", [128, 8], f32, kind="ExternalInput")
    outT = nc.dram_tensor("outT",
                          [128 if small_out else C, 512 if small_out else N],
                          f32, kind="ExternalOutput")

    EXPF = mybir.ActivationFunctionType.Exp

    with tile.TileContext(nc) as tc:
        # All pools live for the whole program; per-rep tiles rotate via
        # tags so cross-rep dependencies are per-tensor WAR (a per-rep pool
        # close would barrier rep r+1's loads on ALL of rep r).
        with tc.tile_pool(name="persist", bufs=1) as pp, \
             tc.tile_pool(name="wts", bufs=1) as wp, \
             tc.tile_pool(name="bwork", bufs=1) as bw, \
             tc.tile_pool(name="etp", bufs=2) as etp, \
             tc.tile_pool(name="ostp", bufs=1) as osp, \
             tc.tile_pool(name="ps", bufs=1, space="PSUM") as ps:
            qt_sb = [pp.tile([128, N], fr, name=f"qt{t}") for t in range(2)]
            # per-head zero-padded K: even h -> rows 0:64, odd h -> 64:128
            kt_pad = [pp.tile([128, N], fr, name=f"ktp{h}")
                      for h in range(HPC)]
            for h in range(HPC):
                z = slice(64, 128) if h % 2 == 0 else slice(0, 64)
                nc.vector.memset(kt_pad[h][z, :].bitcast(mybir.dt.uint32), 0)
            att_sb = [pp.tile([128, N], fr, name=f"att{t}") for t in range(2)]
            v_sb = pp.tile([128, RT_TILES, V_COLS], fr, name="v_sb")
            for base in (64, 256):   # ones + zero columns of the V panels
                nc.vector.memset(
                    v_sb[:, :, base].bitcast(mybir.dt.uint32), 0x3F800000)
                nc.vector.memset(
                    v_sb[:, :, base + 1:base + 64].bitcast(mybir.dt.uint32),
                    0)
            ones = pp.tile([128, 128], fr, name="ones")
            nc.vector.memset(ones[:].bitcast(mybir.dt.uint32), 0x3F800000)

            ctx = _Ctx(nc=nc, ds=ds, fr=fr, f32=f32, EXPF=EXPF,
                       IDENT=mybir.ActivationFunctionType.Identity,
                       wp=wp, bw=bw, etp=etp, osp=osp, ps=ps, ones=ones,
                       qt_sb=qt_sb, kt_pad=kt_pad, att_sb=att_sb,
                       v_sb=v_sb, outT=outT, small_out=small_out,
                       dram=dict(xt=xt_d, wq=wq_d, wk=wk_d, wv=wv_d,
                                 wo=wo_d, bqk=bqk_d, bv=bv_d, bo4=bo4_d))
            for rep in range(nrep):
                T = _emit_loads(ctx)
                _emit_A(ctx, T)
                if phases == "A":
                    _dbg_out(ctx, kt_pad[0])
                    break
                _emit_B(ctx, T)
                if phases == "AB":
                    _dbg_out(ctx, att_sb[0])
                    break
                _emit_C(ctx, T)
    nc.compile()
    return nc


class _Ctx:
    def __init__(self, **kw):
        self.__dict__.update(kw)


def _dbg_out(ctx, src):
    d = ctx.osp.tile([128, N], ctx.f32, name="ost", tag="ost")
    ctx.nc.vector.tensor_copy(d[:, 0:512], src[:, 0:512])
    ctx.nc.sync.dma_start(out=ctx.outT[0:128, 0:512], in_=d[:, 0:512])


def _emit_loads(ctx):
    nc, fr, f32 = ctx.nc, ctx.fr, ctx.f32
    wp, d = ctx.wp, ctx.dram
    T = {}
    T["xt"] = wp.tile([128, KT_TILES * N], fr, name="xt", tag="xt")
    nc.gpsimd.dma_start(out=T["xt"][:], in_=d["xt"][:])
    T["wq"] = wp.tile([128, KT_TILES * CHC], fr, name="wq", tag="wq")
    nc.gpsimd.dma_start(out=T["wq"][:], in_=d["wq"][:])
    T["wk"] = wp.tile([128, KT_TILES * CHC], fr, name="wk", tag="wk")
    nc.gpsimd.dma_start(out=T["wk"][:], in_=d["wk"][:])
    T["wv"] = wp.tile([128, KT_TILES * CHC], fr, name="wv", tag="wv")
    nc.gpsimd.dma_start(out=T["wv"][:], in_=d["wv"][:])
    T["bqk"] = wp.tile([128, 4], f32, name="bqk", tag="bqk")
    nc.gpsimd.dma_start(out=T["bqk"][:], in_=d["bqk"][:])
    T["bv"] = wp.tile([1, CHC], fr, name="bv", tag="bv")
    nc.gpsimd.dma_start(out=T["bv"][:], in_=d["bv"][:])
    T["bo4"] = wp.tile([128, 8], f32, name="bo4", tag="bo4")
    nc.gpsimd.dma_start(out=T["bo4"][:], in_=d["bo4"][:])
    # wo is read until the end of phase C; its WAR dep would block the
    # Pool queue (and everything behind it) until then — keep it on SP.
    T["wo"] = wp.tile([128, 2 * C], fr, name="wo", tag="wo")
    nc.sync.dma_start(out=T["wo"][:], in_=d["wo"][:])
    return T


def _emit_A(ctx, T):
    nc, ds, fr, f32 = ctx.nc, ctx.ds, ctx.fr, ctx.f32

    # bv broadcast tile [128, 256] via a PE ones-matmul
    bvb_ps = ctx.ps.tile([128, 512], f32, name="bvb_ps", tag="p512", bufs=4)
    nc.tensor.matmul(bvb_ps[:, 0:CHC], ctx.ones[0:1, 0:128], T["bv"][0:1, :],
                     start=True, stop=True)
    bvb = ctx.wp.tile([128, CHC], f32, name="bvb", tag="bvb")
    nc.vector.tensor_copy(bvb[:], bvb_ps[:, 0:CHC])

    # QT / K(padded): 4 query-chunk accumulators per weight slice so that
    # consecutive matmuls share the stationary operand
    for wmat, bcol in (("wq", 0), ("wk", 2)):
        for t in range(2):
            accs = [ctx.ps.tile([128, 512], f32, name="acc", tag="p512",
                                bufs=4) for _ in range(4)]
            for k in range(KT_TILES):
                for qc in range(4):
                    nc.tensor.matmul(
                        accs[qc][:], T[wmat][:, ds(k * CHC + t * 128, 128)],
                        T["xt"][:, ds(k * N + qc * 512, 512)],
                        start=(k == 0), stop=(k == KT_TILES - 1))
            for qc in range(4):
                if wmat == "wq":
                    nc.vector.tensor_scalar_add(
                        ctx.qt_sb[t][:, ds(qc * 512, 512)], accs[qc][:],
                        T["bqk"][:, bcol + t:bcol + t + 1])
                else:
                    # split the drain per head into the zero-padded tiles
                    for i, h in enumerate((2 * t, 2 * t + 1)):
                        rows = slice(64 * i, 64 * i + 64)
                        nc.vector.tensor_scalar_add(
                            ctx.kt_pad[h][rows, ds(qc * 512, 512)],
                            accs[qc][rows, :],
                            T["bqk"][rows, bcol + t:bcol + t + 1])

    # V in keys-major layout: V[keys, ch] = xT^T @ Wv (+bv)
    for kt in range(RT_TILES):
        acc = ctx.ps.tile([128, 512], f32, name="vacc", tag="p512", bufs=4)
        for k in range(KT_TILES):
            nc.tensor.matmul(
                acc[:, 0:CHC], T["xt"][:, ds(k * N + kt * 128, 128)],
                T["wv"][:, ds(k * CHC, CHC)],
                start=(k == 0), stop=(k == KT_TILES - 1))
        for h in range(HPC):
            nc.vector.tensor_add(
                ctx.v_sb[:, kt, ds(V_STORE[h], 64)],
                acc[:, ds(h * 64, 64)], bvb[:, ds(h * 64, 64)])


def _emit_B(ctx, T):
    nc, ds, fr, f32 = ctx.nc, ctx.ds, ctx.fr, ctx.f32
    qt_sb, kt_pad, att_sb, v_sb = (ctx.qt_sb, ctx.kt_pad, ctx.att_sb,
                                   ctx.v_sb)
    pending = [None]

    def make_norm(tI, pO, qc, av, even):
        def norm():
            dr = 64 if even else 0   # denominator row
            rs = ctx.bw.tile([128, 512], fr, name="rs", tag="rs")
            with nc.allow_low_precision("f32r softmax denom"):
                nc.vector.reciprocal(rs[dr:dr + 1, :], av[dr:dr + 1, :])
            # broadcast 1/denom across partitions via a PE ones-matmul
            # (dst must start at partition 0: broadcast all 128 rows)
            bc = ctx.ps.tile([128, 512], f32, name="bc", tag="p512", bufs=4)
            nc.tensor.matmul(bc[:], ctx.ones[dr:dr + 1, 0:128],
                             rs[dr:dr + 1, :], start=True, stop=True)
            bcs = ctx.bw.tile([128, 512], f32, name="bcs", tag="bcs")
            nc.vector.tensor_copy(bcs[pO:pO + 64, :], bc[pO:pO + 64, :])
            nc.vector.tensor_mul(
                att_sb[tI][pO:pO + 64, ds(qc * 512, 512)],
                av[pO:pO + 64, :], bcs[pO:pO + 64, :])
        return norm

    for h in range(HPC):
        tI, pO = h // 2, 64 * (h % 2)
        even = (h % 2 == 0)
        lb, lw = V_LHS[h]
        for qp in range(2):
            qA, qB = 2 * qp, 2 * qp + 1
            avs = []
            ets = [None] * RT_TILES

            def emit_av(g, avs=avs, lb=lb, lw=lw, ets=ets):
                for j in range(2):   # consecutive MMs share the V panel
                    nc.tensor.matmul(
                        avs[j][0:lw, :], v_sb[:, g, ds(lb, lw)],
                        ets[g][:, ds(j * 512, 512)],
                        start=(g == 0), stop=(g == RT_TILES - 1))

            for g in range(RT_TILES):
                sc = ctx.ps.tile([128, 1024], f32, name="sc", tag="sc",
                                 bufs=2)
                for j, qc in enumerate((qA, qB)):
                    # full-128 contraction: kt_pad's zero rows null the
                    # other head's Q rows
                    nc.tensor.matmul(
                        sc[:, ds(j * 512, 512)],
                        kt_pad[h][:, ds(g * 128, 128)],
                        qt_sb[tI][:, ds(qc * 512, 512)],
                        start=True, stop=True)
                et = ctx.etp.tile([128, 1024], fr, name="et", tag="et")
                nc.scalar.activation(et[:], sc[:], ctx.EXPF, bias=0.0,
                                     scale=0.125)
                ets[g] = et
                if g == 1:
                    if pending[0] is not None:
                        pending[0]()
                        pending[0] = None
                    avs.extend(ctx.ps.tile([128, 512], f32, name="av",
                                           tag="p512", bufs=4)
                               for _ in range(2))
                if g >= 1:
                    emit_av(g - 1)
            emit_av(RT_TILES - 1)

            def both(na=make_norm(tI, pO, qA, avs[0], even),
                     nb=make_norm(tI, pO, qB, avs[1], even)):
                na()
                nb()
            pending[0] = both
    pending[0]()


def _emit_C(ctx, T):
    # C accumulators use the "sc" psum tag; each wo slice is stationary
    # for 4 consecutive matmuls (query chunks)
    nc, ds, f32 = ctx.nc, ctx.ds, ctx.f32
    for m in range(8):
        ost = ctx.osp.tile([128, N], f32, name="ost", tag="ost")
        accs = [ctx.ps.tile([128, 1024], f32, name="cacc", tag="sc",
                            bufs=2) for _ in range(2)]
        for t in range(2):
            for qc in range(4):
                nc.tensor.matmul(
                    accs[qc // 2][:, ds((qc % 2) * 512, 512)],
                    T["wo"][:, ds(t * C + m * 128, 128)],
                    ctx.att_sb[t][:, ds(qc * 512, 512)],
                    start=(t == 0), stop=(t == 1))
        # alternate drains between DVE and ACT so neither gates the PE
        nc.vector.tensor_scalar_add(ost[:, 0:1024], accs[0][:],
                                    T["bo4"][:, m:m + 1])
        nc.scalar.activation(ost[:, 1024:2048], accs[1][:], ctx.IDENT,
                             bias=T["bo4"][:, m:m + 1], scale=1.0)
        if ctx.small_out:
            if m == 0:
                nc.sync.dma_start(out=ctx.outT[:, :], in_=ost[:, 0:512])
        else:
            nc.sync.dma_start(out=ctx.outT[ds(m * 128, 128), :], in_=ost[:])


# ---------------------------------------------------------------------------
# Host-side: runner (one-time jit) + kernel() entry point
# ---------------------------------------------------------------------------

class _SpmdRunner:
    def __init__(self, nc, n_cores=NCORES):
        import jax
        import numpy as _np
        from jax.sharding import Mesh, PartitionSpec
        from jax.experimental.shard_map import shard_map
        import concourse.mybir as mybir
        from concourse import bass2jax
        from concourse.bass2jax import _bass_exec_p, install_neuronx_cc_hook

        install_neuronx_cc_hook()
        self.jax = jax
        self.n_cores = n_cores
        partition_name = (nc.partition_id_tensor.name
                          if nc.partition_id_tensor else None)
        in_names, out_names, out_avals, zero_outs = [], [], [], []
        for alloc in nc.m.functions[0].allocations:
            if not isinstance(alloc, mybir.MemoryLocationSet):
                continue
            name = alloc.memorylocations[0].name
            if alloc.kind == "ExternalInput":
                if name != partition_name:
                    in_names.append(name)
            elif alloc.kind == "ExternalOutput":
                out_names.append(name)
                shape = tuple(alloc.tensor_shape)
                dtype = mybir.dt.np(alloc.dtype)
                out_avals.append(jax.core.ShapedArray(shape, dtype))
                zero_outs.append(_np.zeros(shape, dtype))
        self.in_names, self.out_names = in_names, out_names
        self.out_avals, self.zero_outs = out_avals, zero_outs
        n_params, n_outs = len(in_names), len(out_names)
        all_in = list(in_names) + list(out_names)
        if partition_name is not None:
            all_in.append(partition_name)
        donate = tuple(range(n_params, n_params + n_outs))

        def _body(*args):
            operands = list(args)
            if partition_name is not None:
                operands.append(bass2jax.partition_id_tensor())
            outs = _bass_exec_p.bind(
                *operands, out_avals=tuple(out_avals),
                in_names=tuple(all_in), out_names=tuple(out_names),
                lowering_input_output_aliases=(),
                sim_require_finite=True, sim_require_nnan=True, nc=nc)
            return tuple(outs)

        devices = jax.devices()[:n_cores]
        self.mesh = Mesh(_np.asarray(devices), ("core",))
        in_specs = (PartitionSpec("core"),) * (n_params + n_outs)
        out_specs = (PartitionSpec("core"),) * n_outs
        self.sharded = jax.jit(
            shard_map(_body, mesh=self.mesh, in_specs=in_specs,
                      out_specs=out_specs, check_rep=False),
            donate_argnums=donate, keep_unused=True)
        self._PartitionSpec = PartitionSpec

    def set_inputs(self, in_maps):
        import jax
        from jax.sharding import NamedSharding
        per_core = [[np.asarray(m[name]) for name in self.in_names]
                    for m in in_maps]
        sharding = NamedSharding(self.mesh, self._PartitionSpec("core"))
        self._in = [
            jax.device_put(np.concatenate(
                [per_core[c][i] for c in range(self.n_cores)], axis=0),
                sharding)
            for i in range(len(self.in_names))
        ]
        jax.block_until_ready(self._in)

    def run(self):
        import jax
        zeros = [np.zeros((self.n_cores * z.shape[0], *z.shape[1:]), z.dtype)
                 for z in self.zero_outs]
        out = self.sharded(*self._in, *zeros)
        jax.block_until_ready(out)
        return out

    def results(self, out_arrs):
        return [
            {name: np.asarray(out_arrs[i]).reshape(
                self.n_cores, *self.out_avals[i].shape)[c]
             for i, name in enumerate(self.out_names)}
            for c in range(self.n_cores)
        ]


def _get_runner(nrep: int = 1):
    key = ("runner", nrep, os.environ.get("MHA_KDTYPE", "f32r"))
    if key not in _CACHE:
        nc = _build_nc(nrep=nrep, kdtype=os.environ.get("MHA_KDTYPE", "f32r"))
        _CACHE[key] = _SpmdRunner(nc)
    return _CACHE[key]


def _make_in_maps(x, Wq, bq, Wk, bk, Wv, bv, Wo, bo):
    wq_f = np.asarray(Wq, np.float32)
    wk_f = np.asarray(Wk, np.float32)
    wv_f = np.asarray(Wv, np.float32)
    wo_f = np.asarray(Wo, np.float32)
    bq_f = np.asarray(bq, np.float32)
    bk_f = np.asarray(bk, np.float32)
    bv_f = np.asarray(bv, np.float32)
    bo_f = np.asarray(bo, np.float32)
    x_f = np.asarray(x, np.float32)

    # xt host tiling: xt[p, k*N + n] = x[b][n, k*128+p]
    xts = []
    for b in range(B):
        xT = x_f[b].T                                  # [C, N]
        xts.append(np.ascontiguousarray(
            xT.reshape(KT_TILES, 128, N).transpose(1, 0, 2).reshape(128, -1)))

    bo_t = np.ascontiguousarray(bo_f.reshape(8, 128).T) / 4.0

    in_maps = []
    for c in range(NCORES):
        b, hg = c // HPC, c % HPC
        ch = slice(CHC * hg, CHC * (hg + 1))
        # w[p, k*CHC + j] = W[k*128+p, ch0+j]
        wqc = np.ascontiguousarray(
            wq_f[:, ch].reshape(KT_TILES, 128, CHC).transpose(1, 0, 2)
            .reshape(128, -1))
        wkc = np.ascontiguousarray(
            wk_f[:, ch].reshape(KT_TILES, 128, CHC).transpose(1, 0, 2)
            .reshape(128, -1))
        wvc = np.ascontiguousarray(
            wv_f[:, ch].reshape(KT_TILES, 128, CHC).transpose(1, 0, 2)
            .reshape(128, -1))
        # wo[p, t*C + j] = Wo[ch0 + t*128 + p, j]
        woc = np.ascontiguousarray(
            wo_f[ch, :].reshape(2, 128, C).transpose(1, 0, 2).reshape(128, -1))
        bqk = np.ascontiguousarray(np.stack(
            [bq_f[ch].reshape(2, 128)[0], bq_f[ch].reshape(2, 128)[1],
             bk_f[ch].reshape(2, 128)[0], bk_f[ch].reshape(2, 128)[1]],
            axis=1))
        in_maps.append({"xt": xts[b], "wq": wqc, "wk": wkc, "wv": wvc,
                        "wo": woc, "bqk": bqk,
                        "bv": np.ascontiguousarray(bv_f[ch].reshape(1, CHC)),
                        "bo4": bo_t})
    return in_maps


def kernel(x, Wq, bq, Wk, bk, Wv, bv, Wo, bo):
    runner = _get_runner()
    runner.set_inputs(_make_in_maps(x, Wq, bq, Wk, bk, Wv, bv, Wo, bo))
    res = runner.results(runner.run())
    out = np.zeros((B, N, C), np.float32)
    for c in range(NCORES):
        b = c // HPC
        out[b] += res[c]["outT"].T
    return out



# revision 12
# speedup vs baseline: 1.5396x; 1.5396x over previous
"""MultiHeadAttention forward on 8 Trainium2 NeuronCores (Bass/Tile).

Problem: B=2, N=2048, C=1024, H=16, D=64, fp32.
  q/k/v = x @ W* + b*; scores = q k^T / sqrt(D); w = softmax(scores);
  out = (w v) @ Wo + bo.

Sharding: tensor-parallel over (batch, head-group). Core c handles batch
b = c//4 and heads 4*(c%4)..4*(c%4)+3 (channel slice of 256). Each core
computes its own Q/K/V projections, attention for its 4 heads, and a
PARTIAL output projection out_part = att @ Wo[ch, :]. The host sums the
4 partials per batch during unshard (row-parallel linear).

Bias identities exploited (exact in real arithmetic):
  - bk is DROPPED: scores[n,m] += q_n . bk is constant per query row n,
    and softmax over keys is invariant to per-row constants.
  - bv is FOLDED into the output bias: softmax weights sum to 1, so
    attention(v + bv) = attention(v) + bv; host bakes bv @ Wo_ch + bo/4
    into the per-core bo4 tile.

HW-calibrated notes (this machine, from perfetto traces):
  - f32r matmul streams 1 col/cycle @2.4GHz (213ns per 512-col matmul);
    LDWEIGHTS (~190-330ns) largely hides under the previous matmul's
    streaming. bf16/fp16 stream at the SAME rate - no dtype win.
  - ACT exp on [128,1024] is 1114ns flat regardless of dst dtype; the
    exp stream (128 tiles/rep) is the phase-B co-bottleneck with PE.
  - DVE ops are free-size-bound (~1.33ns/col); nc.vector.reciprocal is
    ~6.5ns/col (3.3us per [1,512] row!) - use reciprocal_approx_fast
    (~51 ULP, plenty for softmax denominators of O(100..3000)).
  - Every f32r matmul pays its weight load inline (standalone ldweights
    is broken for f32r); keep consecutive matmuls on the same stationary
    operand where convenient, but do not contort the schedule for it.

Per-core pipeline (all matmuls f32r, fp32 PSUM accumulation):
  A: QT[256,2048] = Wq_ch^T @ xT (+bq via ACT/DVE drains); K likewise
     into zero-padded per-head kt tiles (no bias); V[2048,256] in
     keys-major layout with per-head-pair panels [V_even |1| junk |
     V_odd] so each AV matmul also produces the softmax denominator row
     (ones column).
  B: per (head, query-block-pair): for each key tile g: scoresT
     [128,1024] (2 query blocks, shared stationary); et = exp(0.125 s)
     (ACT); AV matmuls run at lag 2 behind the exp stream (ets bufs=3)
     so the PE never waits on ACT latency. Normalization: fast DVE
     reciprocal of the denominator rows right after the last AV, then
     (deferred 2 key tiles into the next block) a PE ones-matmul
     broadcast + one DVE copy + DVE multiplies into att_sb.
  C: out_part^T [1024,2048] = Wo_ch^T @ attT, 4 [128,512] PSUM
     accumulators per output row-block, drained eagerly alternating
     ACT (bias via activation) / DVE (tensor_scalar_add), DMA out.

Host pre-tiles every input so each DMA is one large contiguous
transfer; loads go on the otherwise-idle GPSIMD queue so rep r+1's
transfers hide under rep r's phase B.
"""
import os
import numpy as np

B, N, C, H, D = 2, 2048, 1024, 16, 64
NCORES = 8
HPC = 4              # heads per core
CHC = HPC * D        # channels per core = 256
KT_TILES = 8         # contraction tiles over C
RT_TILES = 16        # key tiles over N

# v_sb per key-tile column layout: two pairs of 192 cols:
#   [V_h0(0:64) | ones(64) | junk(65:128) | V_h1(128:192)]
#   [V_h2(192:256) | ones(256) | junk(257:320) | V_h3(320:384)]
# (the junk region maps to av output rows nobody reads)
V_COLS = 384
V_STORE = (0, 128, 192, 320)   # where head h's V values are stored
# lhsT slices for the AV matmul: even heads [V|1] (65 cols, denom row 64);
# odd heads [1|junk|V] (128 cols, denom row 0, values rows 64..127)
V_LHS = ((0, 65), (64, 128), (192, 65), (256, 128))

_CACHE = {}


def _build_nc(nrep: int = 1, kdtype: str = "f32r", small_out: bool = False,
              phases: str = "ABC", vd_bf16: bool = False):
    import concourse.bacc as bacc
    import concourse.mybir as mybir
    import concourse.tile as tile
    from concourse.bass import ts, ds

    f32 = mybir.dt.float32
    fr = mybir.dt.float32r if kdtype == "f32r" else mybir.dt.float32

    nc = bacc.Bacc("TRN2", target_bir_lowering=False, debug=False)

    # ---- I/O (host pre-tiled; every load is one contiguous DMA) ----
    xt_d = nc.dram_tensor("xt", [128, KT_TILES * N], fr, kind="ExternalInput")
    wq_d = nc.dram_tensor("wq", [128, KT_TILES * CHC], fr, kind="ExternalInput")
    wk_d = nc.dram_tensor("wk", [128, KT_TILES * CHC], fr, kind="ExternalInput")
    wv_d = nc.dram_tensor("wv", [128, KT_TILES * CHC], fr, kind="ExternalInput")
    wo_d = nc.dram_tensor("wo", [128, 2 * C], fr, kind="ExternalInput")
    bq_d = nc.dram_tensor("bq", [128, 2], f32, kind="ExternalInput")
    outT = nc.dram_tensor("outT",
                          [128 if small_out else C, 512 if small_out else N],
                          f32, kind="ExternalOutput")

    EXPF = mybir.ActivationFunctionType.Exp

    with tile.TileContext(nc) as tc:
        # All pools live for the whole program; per-rep tiles rotate via
        # tags so cross-rep dependencies are per-tensor WAR (a per-rep pool
        # close would barrier rep r+1's loads on ALL of rep r).
        with tc.tile_pool(name="persist", bufs=1) as pp, \
             tc.tile_pool(name="wts", bufs=1) as wp, \
             tc.tile_pool(name="bwork", bufs=1) as bw, \
             tc.tile_pool(name="etp", bufs=3) as etp, \
             tc.tile_pool(name="ostp", bufs=1) as osp, \
             tc.tile_pool(name="ps", bufs=1, space="PSUM") as ps:
            qt_sb = [pp.tile([128, N], fr, name=f"qt{t}") for t in range(2)]
            # per-head zero-padded K: even h -> rows 0:64, odd h -> 64:128
            kt_pad = [pp.tile([128, N], fr, name=f"ktp{h}")
                      for h in range(HPC)]
            for h in range(HPC):
                z = slice(64, 128) if h % 2 == 0 else slice(0, 64)
                nc.vector.memset(kt_pad[h][z, :].bitcast(mybir.dt.uint32), 0)
            att_sb = [pp.tile([128, N], fr, name=f"att{t}") for t in range(2)]
            v_sb = pp.tile([128, RT_TILES, V_COLS], fr, name="v_sb")
            for base in (64, 256):   # ones + zero columns of the V panels
                nc.vector.memset(
                    v_sb[:, :, base].bitcast(mybir.dt.uint32), 0x3F800000)
                nc.vector.memset(
                    v_sb[:, :, base + 1:base + 64].bitcast(mybir.dt.uint32),
                    0)
            ones = pp.tile([128, 128], fr, name="ones")
            nc.vector.memset(ones[:].bitcast(mybir.dt.uint32), 0x3F800000)

            ctx = _Ctx(nc=nc, ds=ds, fr=fr, f32=f32, EXPF=EXPF,
                       IDENT=mybir.ActivationFunctionType.Identity,
                       wp=wp, bw=bw, etp=etp, osp=osp, ps=ps, ones=ones,
                       qt_sb=qt_sb, kt_pad=kt_pad, att_sb=att_sb,
                       v_sb=v_sb, outT=outT, small_out=small_out,
                       dram=dict(xt=xt_d, wq=wq_d, wk=wk_d, wv=wv_d,
                                 wo=wo_d, bq=bq_d))
            for rep in range(nrep):
                T = _emit_loads(ctx)
                _emit_A(ctx, T)
                if phases == "A":
                    _dbg_out(ctx, kt_pad[0])
                    break
                _emit_B(ctx, T)
                if phases == "AB":
                    _dbg_out(ctx, att_sb[0])
                    break
                _emit_C(ctx, T)
    nc.compile()
    return nc


class _Ctx:
    def __init__(self, **kw):
        self.__dict__.update(kw)


def _dbg_out(ctx, src):
    d = ctx.osp.tile([128, N // 2], ctx.f32, name="ost", tag="ost")
    ctx.nc.vector.tensor_copy(d[:, 0:512], src[:, 0:512])
    ctx.nc.sync.dma_start(out=ctx.outT[0:128, 0:512], in_=d[:, 0:512])


def _emit_loads(ctx):
    nc, fr, f32 = ctx.nc, ctx.fr, ctx.f32
    wp, d = ctx.wp, ctx.dram
    T = {}
    T["xt"] = wp.tile([128, KT_TILES * N], fr, name="xt", tag="xt")
    nc.gpsimd.dma_start(out=T["xt"][:], in_=d["xt"][:])
    T["wq"] = wp.tile([128, KT_TILES * CHC], fr, name="wq", tag="wq")
    nc.gpsimd.dma_start(out=T["wq"][:], in_=d["wq"][:])
    T["wk"] = wp.tile([128, KT_TILES * CHC], fr, name="wk", tag="wk")
    nc.gpsimd.dma_start(out=T["wk"][:], in_=d["wk"][:])
    T["wv"] = wp.tile([128, KT_TILES * CHC], fr, name="wv", tag="wv")
    nc.gpsimd.dma_start(out=T["wv"][:], in_=d["wv"][:])
    T["bq"] = wp.tile([128, 2], f32, name="bq", tag="bq")
    nc.gpsimd.dma_start(out=T["bq"][:], in_=d["bq"][:])
    # wo is read until the end of phase C; its WAR dep would block the
    # Pool queue (and everything behind it) until then — keep it on SP.
    T["wo"] = wp.tile([128, 2 * C], fr, name="wo", tag="wo")
    nc.sync.dma_start(out=T["wo"][:], in_=d["wo"][:])
    return T


def _emit_A(ctx, T):
    nc, ds, fr, f32 = ctx.nc, ctx.ds, ctx.fr, ctx.f32

    # QT / K: 4 query-chunk accumulators per weight slice so that
    # consecutive matmuls share the stationary operand
    for wmat in ("wq", "wk"):
        for t in range(2):
            accs = [ctx.ps.tile([128, 512], f32, name="acc", tag="p512",
                                bufs=4) for _ in range(4)]
            for k in range(KT_TILES):
                for qc in range(4):
                    nc.tensor.matmul(
                        accs[qc][:], T[wmat][:, ds(k * CHC + t * 128, 128)],
                        T["xt"][:, ds(k * N + qc * 512, 512)],
                        start=(k == 0), stop=(k == KT_TILES - 1))
            # drains alternate ACT/DVE so neither engine gates psum reuse
            for qc in range(4):
                if wmat == "wq":
                    if qc % 2 == 0:
                        nc.scalar.activation(
                            ctx.qt_sb[t][:, ds(qc * 512, 512)], accs[qc][:],
                            ctx.IDENT, bias=T["bq"][:, t:t + 1], scale=1.0)
                    else:
                        nc.vector.tensor_scalar_add(
                            ctx.qt_sb[t][:, ds(qc * 512, 512)], accs[qc][:],
                            T["bq"][:, t:t + 1])
                else:
                    # plain copies into the zero-padded per-head kt tiles
                    for i, h in enumerate((2 * t, 2 * t + 1)):
                        rows = slice(64 * i, 64 * i + 64)
                        if i == 0:
                            nc.scalar.activation(
                                ctx.kt_pad[h][rows, ds(qc * 512, 512)],
                                accs[qc][rows, :], ctx.IDENT, bias=0.0,
                                scale=1.0)
                        else:
                            nc.vector.tensor_copy(
                                ctx.kt_pad[h][rows, ds(qc * 512, 512)],
                                accs[qc][rows, :])

    # V in keys-major layout: V[keys, ch] = xT^T @ Wv (no bias: folded)
    for kt in range(RT_TILES):
        acc = ctx.ps.tile([128, 512], f32, name="vacc", tag="p512", bufs=4)
        for k in range(KT_TILES):
            nc.tensor.matmul(
                acc[:, 0:CHC], T["xt"][:, ds(k * N + kt * 128, 128)],
                T["wv"][:, ds(k * CHC, CHC)],
                start=(k == 0), stop=(k == KT_TILES - 1))
        for h in range(HPC):
            nc.vector.tensor_copy(
                ctx.v_sb[:, kt, ds(V_STORE[h], 64)], acc[:, ds(h * 64, 64)])


def _emit_B(ctx, T):
    nc, ds, fr, f32 = ctx.nc, ctx.ds, ctx.fr, ctx.f32
    qt_sb, kt_pad, att_sb, v_sb = (ctx.qt_sb, ctx.kt_pad, ctx.att_sb,
                                   ctx.v_sb)
    G = RT_TILES
    pending = [None]

    for h in range(HPC):
        tI, pO = h // 2, 64 * (h % 2)
        even = (h % 2 == 0)
        dr = 64 if even else 0   # denominator row in the av psum
        lb, lw = V_LHS[h]
        for qp in range(2):
            qA, qB = 2 * qp, 2 * qp + 1
            avs = []
            ets = [None] * G

            def emit_av(g, avs=avs, lb=lb, lw=lw, ets=ets):
                for j in range(2):   # consecutive MMs share the V panel
                    nc.tensor.matmul(
                        avs[j][0:lw, :], v_sb[:, g, ds(lb, lw)],
                        ets[g][:, ds(j * 512, 512)],
                        start=(g == 0), stop=(g == G - 1))

            for g in range(G):
                sc = ctx.ps.tile([128, 1024], f32, name="sc", tag="sc",
                                 bufs=2)
                for j, qc in enumerate((qA, qB)):
                    # full-128 contraction: kt_pad's zero rows null the
                    # other head's Q rows
                    nc.tensor.matmul(
                        sc[:, ds(j * 512, 512)],
                        kt_pad[h][:, ds(g * 128, 128)],
                        qt_sb[tI][:, ds(qc * 512, 512)],
                        start=True, stop=True)
                et = ctx.etp.tile([128, 1024], fr, name="et", tag="et")
                nc.scalar.activation(et[:], sc[:], ctx.EXPF, bias=0.0,
                                     scale=0.125)
                ets[g] = et
                if g == 1:
                    avs.extend(ctx.ps.tile([128, 512], f32, name="av",
                                           tag="p512", bufs=4)
                               for _ in range(2))
                if g == 2 and pending[0] is not None:
                    pending[0]()
                    pending[0] = None
                if g >= 2:         # AV runs at lag 2 behind the exp stream
                    emit_av(g - 2)
            emit_av(G - 2)
            emit_av(G - 1)

            # fast reciprocals of the two denominator rows, immediately
            # (DVE, ~51 ULP; denominators are sums of 2048 positive exps).
            # nrm row dr holds 1/denom; rows pO:pO+64 (disjoint) later hold
            # the PE-broadcast copy - one shared SBUF tile.
            nrm = ctx.bw.tile([128, 1024], f32, name="nrm", tag="nrm")
            from concourse.dve_ops import (RECIP_APPROX_FAST_CONSTS,
                                           RECIPROCAL_APPROX_FAST)
            for j in range(2):
                if dr == 0:
                    # fast approx recip (~51 ULP, ~5x) - but custom-DVE ops
                    # give wrong results at partition offsets > 0 on HW, so
                    # only the odd heads (denominator row 0) may use it
                    cst = RECIP_APPROX_FAST_CONSTS
                    nc.vector._custom_dve(
                        RECIPROCAL_APPROX_FAST,
                        out=nrm[dr:dr + 1, ds(j * 512, 512)].bitcast(ctx.fr),
                        in0=avs[j][dr:dr + 1, :],
                        s0=cst["s0"], s1=cst["s1"], imm2=cst["imm2"])
                else:
                    # even heads: native reciprocal (offset-safe; 3.3us on
                    # DVE but off the PE critical path - the consuming bc
                    # matmul is deferred a whole block)
                    with nc.allow_low_precision("f32r softmax denom"):
                        nc.vector.reciprocal(
                            nrm[dr:dr + 1, ds(j * 512, 512)].bitcast(ctx.fr),
                            avs[j][dr:dr + 1, :])

            def finish(avs=avs, nrm=nrm, tI=tI, pO=pO, dr=dr, qA=qA, qB=qB):
                # PE broadcast of 1/denom rows (rides the "sc" psum tag so
                # the av psum rotation is reuse-distance 2 blocks)
                bc = ctx.ps.tile([128, 1024], f32, name="bc", tag="sc",
                                 bufs=2)
                for j in range(2):
                    nc.tensor.matmul(bc[:, ds(j * 512, 512)],
                                     ctx.ones[dr:dr + 1, 0:128],
                                     nrm[dr:dr + 1, ds(j * 512, 512)]
                                     .bitcast(ctx.fr),
                                     start=True, stop=True)
                # f32r-typed dst: the verifier tracks rounding per memory
                # location, and nrm's row dr feeds an f32r matmul
                nc.vector.tensor_copy(nrm[pO:pO + 64, :].bitcast(ctx.fr),
                                      bc[pO:pO + 64, :])
                for j, qc in enumerate((qA, qB)):
                    nc.vector.tensor_mul(
                        att_sb[tI][pO:pO + 64, ds(qc * 512, 512)],
                        avs[j][pO:pO + 64, :],
                        nrm[pO:pO + 64, ds(j * 512, 512)])
            pending[0] = finish
    pending[0]()


def _emit_C(ctx, T):
    # 4 [128,512] accumulators per output row-block (p512 tag), drained
    # eagerly (plain copies - the output bias is added on the host while
    # unsharding) and DMA'd out in [128,1024] halves
    nc, ds, f32 = ctx.nc, ctx.ds, ctx.f32
    for m in range(8):
        for half in range(2):
            ost = ctx.osp.tile([128, N // 2], f32, name="ost", tag="ost")
            for j in range(2):
                qc = 2 * half + j
                acc = ctx.ps.tile([128, 512], f32, name="cacc", tag="p512",
                                  bufs=4)
                for t in range(2):
                    nc.tensor.matmul(
                        acc[:], T["wo"][:, ds(t * C + m * 128, 128)],
                        ctx.att_sb[t][:, ds(qc * 512, 512)],
                        start=(t == 0), stop=(t == 1))
                if j == 0:
                    nc.scalar.activation(ost[:, ds(j * 512, 512)], acc[:],
                                         ctx.IDENT, bias=0.0, scale=1.0)
                else:
                    nc.vector.tensor_copy(ost[:, ds(j * 512, 512)], acc[:])
            if ctx.small_out:
                if m == 0 and half == 0:
                    nc.sync.dma_start(out=ctx.outT[:, :], in_=ost[:, 0:512])
            else:
                nc.sync.dma_start(
                    out=ctx.outT[ds(m * 128, 128), ds(half * 1024, 1024)],
                    in_=ost[:])


# ---------------------------------------------------------------------------
# Host-side: runner (one-time jit) + kernel() entry point
# ---------------------------------------------------------------------------

class _SpmdRunner:
    def __init__(self, nc, n_cores=NCORES):
        import jax
        import numpy as _np
        from jax.sharding import Mesh, PartitionSpec
        from jax.experimental.shard_map import shard_map
        import concourse.mybir as mybir
        from concourse import bass2jax
        from concourse.bass2jax import _bass_exec_p, install_neuronx_cc_hook

        install_neuronx_cc_hook()
        self.jax = jax
        self.n_cores = n_cores
        partition_name = (nc.partition_id_tensor.name
                          if nc.partition_id_tensor else None)
        in_names, out_names, out_avals, zero_outs = [], [], [], []
        for alloc in nc.m.functions[0].allocations:
            if not isinstance(alloc, mybir.MemoryLocationSet):
                continue
            name = alloc.memorylocations[0].name
            if alloc.kind == "ExternalInput":
                if name != partition_name:
                    in_names.append(name)
            elif alloc.kind == "ExternalOutput":
                out_names.append(name)
                shape = tuple(alloc.tensor_shape)
                dtype = mybir.dt.np(alloc.dtype)
                out_avals.append(jax.core.ShapedArray(shape, dtype))
                zero_outs.append(_np.zeros(shape, dtype))
        self.in_names, self.out_names = in_names, out_names
        self.out_avals, self.zero_outs = out_avals, zero_outs
        n_params, n_outs = len(in_names), len(out_names)
        all_in = list(in_names) + list(out_names)
        if partition_name is not None:
            all_in.append(partition_name)
        donate = tuple(range(n_params, n_params + n_outs))

        def _body(*args):
            operands = list(args)
            if partition_name is not None:
                operands.append(bass2jax.partition_id_tensor())
            outs = _bass_exec_p.bind(
                *operands, out_avals=tuple(out_avals),
                in_names=tuple(all_in), out_names=tuple(out_names),
                lowering_input_output_aliases=(),
                sim_require_finite=True, sim_require_nnan=True, nc=nc)
            return tuple(outs)

        devices = jax.devices()[:n_cores]
        self.mesh = Mesh(_np.asarray(devices), ("core",))
        in_specs = (PartitionSpec("core"),) * (n_params + n_outs)
        out_specs = (PartitionSpec("core"),) * n_outs
        self.sharded = jax.jit(
            shard_map(_body, mesh=self.mesh, in_specs=in_specs,
                      out_specs=out_specs, check_rep=False),
            donate_argnums=donate, keep_unused=True)
        self._PartitionSpec = PartitionSpec

    def set_inputs(self, in_maps):
        import jax
        from jax.sharding import NamedSharding
        per_core = [[np.asarray(m[name]) for name in self.in_names]
                    for m in in_maps]
        sharding = NamedSharding(self.mesh, self._PartitionSpec("core"))
        self._in = [
            jax.device_put(np.concatenate(
                [per_core[c][i] for c in range(self.n_cores)], axis=0),
                sharding)
            for i in range(len(self.in_names))
        ]
        jax.block_until_ready(self._in)

    def run(self):
        import jax
        zeros = [np.zeros((self.n_cores * z.shape[0], *z.shape[1:]), z.dtype)
                 for z in self.zero_outs]
        out = self.sharded(*self._in, *zeros)
        jax.block_until_ready(out)
        return out

    def results(self, out_arrs):
        return [
            {name: np.asarray(out_arrs[i]).reshape(
                self.n_cores, *self.out_avals[i].shape)[c]
             for i, name in enumerate(self.out_names)}
            for c in range(self.n_cores)
        ]


def _get_runner(nrep: int = 1):
    key = ("runner", nrep, os.environ.get("MHA_KDTYPE", "f32r"))
    if key not in _CACHE:
        nc = _build_nc(nrep=nrep, kdtype=os.environ.get("MHA_KDTYPE", "f32r"))
        _CACHE[key] = _SpmdRunner(nc)
    return _CACHE[key]


def _make_in_maps(x, Wq, bq, Wk, bk, Wv, bv, Wo, bo):
    wq_f = np.asarray(Wq, np.float32)
    wk_f = np.asarray(Wk, np.float32)
    wv_f = np.asarray(Wv, np.float32)
    wo_f = np.asarray(Wo, np.float32)
    bq_f = np.asarray(bq, np.float32)
    bv_f = np.asarray(bv, np.float32)
    bo_f = np.asarray(bo, np.float32)
    x_f = np.asarray(x, np.float32)

    # xt host tiling: xt[p, k*N + n] = x[b][n, k*128+p]
    xts = []
    for b in range(B):
        xT = x_f[b].T                                  # [C, N]
        xts.append(np.ascontiguousarray(
            xT.reshape(KT_TILES, 128, N).transpose(1, 0, 2).reshape(128, -1)))

    in_maps = []
    for c in range(NCORES):
        b, hg = c // HPC, c % HPC
        ch = slice(CHC * hg, CHC * (hg + 1))
        # w[p, k*CHC + j] = W[k*128+p, ch0+j]
        wqc = np.ascontiguousarray(
            wq_f[:, ch].reshape(KT_TILES, 128, CHC).transpose(1, 0, 2)
            .reshape(128, -1))
        wkc = np.ascontiguousarray(
            wk_f[:, ch].reshape(KT_TILES, 128, CHC).transpose(1, 0, 2)
            .reshape(128, -1))
        wvc = np.ascontiguousarray(
            wv_f[:, ch].reshape(KT_TILES, 128, CHC).transpose(1, 0, 2)
            .reshape(128, -1))
        # wo[p, t*C + j] = Wo[ch0 + t*128 + p, j]
        woc = np.ascontiguousarray(
            wo_f[ch, :].reshape(2, 128, C).transpose(1, 0, 2).reshape(128, -1))
        bqc = np.ascontiguousarray(
            np.stack([bq_f[ch].reshape(2, 128)[0],
                      bq_f[ch].reshape(2, 128)[1]], axis=1))
        in_maps.append({"xt": xts[b], "wq": wqc, "wk": wkc, "wv": wvc,
                        "wo": woc, "bq": bqc})
    return in_maps


def kernel(x, Wq, bq, Wk, bk, Wv, bv, Wo, bo):
    runner = _get_runner()
    runner.set_inputs(_make_in_maps(x, Wq, bq, Wk, bk, Wv, bv, Wo, bo))
    res = runner.results(runner.run())
    out = np.zeros((B, N, C), np.float32)
    for c in range(NCORES):
        b = c // HPC
        out[b] += res[c]["outT"].T
    # output bias, plus the folded V bias (softmax weights sum to 1, so
    # attention(v + bv) = attention(v) + bv -> + bv @ Wo on the output)
    bias = (np.asarray(bo, np.float32)
            + np.asarray(bv, np.float32) @ np.asarray(Wo, np.float32))
    out += bias[None, None, :]
    return out


# revision 13
# speedup vs baseline: 1.6622x; 1.0796x over previous
"""MultiHeadAttention forward on 8 Trainium2 NeuronCores (Bass/Tile).

Problem: B=2, N=2048, C=1024, H=16, D=64, fp32.
  q/k/v = x @ W* + b*; scores = q k^T / sqrt(D); w = softmax(scores);
  out = (w v) @ Wo + bo.

Sharding: tensor-parallel over (batch, head-group). Core c handles batch
b = c//4 and heads 4*(c%4)..4*(c%4)+3 (channel slice of 256). Each core
computes its own Q/K/V projections, attention for its 4 heads, and a
PARTIAL output projection out_part = att @ Wo[ch, :]. The host sums the
4 partials per batch during unshard (row-parallel linear).

Bias identities exploited (exact in real arithmetic):
  - bk is DROPPED: scores[n,m] += q_n . bk is constant per query row n,
    and softmax over keys is invariant to per-row constants.
  - bv is FOLDED into the output bias: softmax weights sum to 1, so
    attention(v + bv) = attention(v) + bv; host bakes bv @ Wo_ch + bo/4
    into the per-core bo4 tile.

HW-calibrated notes (this machine, from perfetto traces):
  - f32r matmul streams 1 col/cycle @2.4GHz (213ns per 512-col matmul);
    LDWEIGHTS (~190-330ns) largely hides under the previous matmul's
    streaming. bf16/fp16 stream at the SAME rate - no dtype win.
  - ACT exp on [128,1024] is 1114ns flat regardless of dst dtype; the
    exp stream (128 tiles/rep) is the phase-B co-bottleneck with PE.
  - DVE ops are free-size-bound (~1.33ns/col); nc.vector.reciprocal is
    ~6.5ns/col (3.3us per [1,512] row!) - use reciprocal_approx_fast
    (~51 ULP, plenty for softmax denominators of O(100..3000)).
  - Every f32r matmul pays its weight load inline (standalone ldweights
    is broken for f32r); keep consecutive matmuls on the same stationary
    operand where convenient, but do not contort the schedule for it.

Per-core pipeline (all matmuls f32r, fp32 PSUM accumulation):
  A: QT[256,2048] = Wq_ch^T @ xT (+bq via ACT/DVE drains); K likewise
     into zero-padded per-head kt tiles (no bias); V[2048,256] in
     keys-major layout with per-head-pair panels [V_even |1| junk |
     V_odd] so each AV matmul also produces the softmax denominator row
     (ones column).
  B: per (head, query-block-pair): for each key tile g: scoresT
     [128,1024] (2 query blocks, shared stationary); et = exp(0.125 s)
     (ACT); AV matmuls run at lag 2 behind the exp stream (ets bufs=3)
     so the PE never waits on ACT latency. Normalization: fast DVE
     reciprocal of the denominator rows right after the last AV, then
     (deferred 2 key tiles into the next block) a PE ones-matmul
     broadcast + one DVE copy + DVE multiplies into att_sb.
  C: out_part^T [1024,2048] = Wo_ch^T @ attT, 4 [128,512] PSUM
     accumulators per output row-block, drained eagerly alternating
     ACT (bias via activation) / DVE (tensor_scalar_add), DMA out.

Host pre-tiles every input so each DMA is one large contiguous
transfer; loads go on the otherwise-idle GPSIMD queue so rep r+1's
transfers hide under rep r's phase B.
"""
import os
import numpy as np

B, N, C, H, D = 2, 2048, 1024, 16, 64
NCORES = 8
HPC = 4              # heads per core
CHC = HPC * D        # channels per core = 256
KT_TILES = 8         # contraction tiles over C
RT_TILES = 16        # key tiles over N

# v_sb per key-tile column layout: two pairs of 192 cols:
#   [V_h0(0:64) | ones(64) | junk(65:128) | V_h1(128:192)]
#   [V_h2(192:256) | ones(256) | junk(257:320) | V_h3(320:384)]
# (the junk region maps to av output rows nobody reads)
V_COLS = 384
V_STORE = (0, 128, 192, 320)   # where head h's V values are stored
# lhsT slices for the AV matmul: even heads [V|1] (65 cols, denom row 64);
# odd heads [1|junk|V] (128 cols, denom row 0, values rows 64..127)
V_LHS = ((0, 65), (64, 128), (192, 65), (256, 128))

_CACHE = {}


def _build_nc(nrep: int = 1, kdtype: str = "f32r", small_out: bool = False,
              phases: str = "ABC", vd_bf16: bool = False):
    import concourse.bacc as bacc
    import concourse.mybir as mybir
    import concourse.tile as tile
    from concourse.bass import ts, ds

    f32 = mybir.dt.float32
    fr = mybir.dt.float32r if kdtype == "f32r" else mybir.dt.float32

    nc = bacc.Bacc("TRN2", target_bir_lowering=False, debug=False)

    # ---- I/O (host pre-tiled; every load is one contiguous DMA) ----
    xt_d = nc.dram_tensor("xt", [128, KT_TILES * N], fr, kind="ExternalInput")
    wq_d = nc.dram_tensor("wq", [128, KT_TILES * CHC], fr, kind="ExternalInput")
    wk_d = nc.dram_tensor("wk", [128, KT_TILES * CHC], fr, kind="ExternalInput")
    wv_d = nc.dram_tensor("wv", [128, KT_TILES * CHC], fr, kind="ExternalInput")
    wo_d = nc.dram_tensor("wo", [128, 2 * C], fr, kind="ExternalInput")
    bq_d = nc.dram_tensor("bq", [128, 2], f32, kind="ExternalInput")
    nrm_d = nc.dram_tensor("nrm_scr", [1, 1024], f32, kind="Internal")
    outT = nc.dram_tensor("outT",
                          [128 if small_out else C, 512 if small_out else N],
                          f32, kind="ExternalOutput")

    EXPF = mybir.ActivationFunctionType.Exp

    with tile.TileContext(nc) as tc:
        # All pools live for the whole program; per-rep tiles rotate via
        # tags so cross-rep dependencies are per-tensor WAR (a per-rep pool
        # close would barrier rep r+1's loads on ALL of rep r).
        with tc.tile_pool(name="persist", bufs=1) as pp, \
             tc.tile_pool(name="wts", bufs=1) as wp, \
             tc.tile_pool(name="bwork", bufs=1) as bw, \
             tc.tile_pool(name="etp", bufs=3) as etp, \
             tc.tile_pool(name="ostp", bufs=1) as osp, \
             tc.tile_pool(name="ps", bufs=1, space="PSUM") as ps:
            qt_sb = [pp.tile([128, N], fr, name=f"qt{t}") for t in range(2)]
            # per-head zero-padded K: even h -> rows 0:64, odd h -> 64:128
            kt_pad = [pp.tile([128, N], fr, name=f"ktp{h}")
                      for h in range(HPC)]
            for h in range(HPC):
                z = slice(64, 128) if h % 2 == 0 else slice(0, 64)
                nc.vector.memset(kt_pad[h][z, :].bitcast(mybir.dt.uint32), 0)
            att_sb = [pp.tile([128, N], fr, name=f"att{t}") for t in range(2)]
            v_sb = pp.tile([128, RT_TILES, V_COLS], fr, name="v_sb")
            for base in (64, 256):   # ones + zero columns of the V panels
                nc.vector.memset(
                    v_sb[:, :, base].bitcast(mybir.dt.uint32), 0x3F800000)
                nc.vector.memset(
                    v_sb[:, :, base + 1:base + 64].bitcast(mybir.dt.uint32),
                    0)
            ctx = _Ctx(nc=nc, ds=ds, fr=fr, f32=f32, EXPF=EXPF,
                       IDENT=mybir.ActivationFunctionType.Identity,
                       wp=wp, bw=bw, etp=etp, osp=osp, ps=ps,
                       qt_sb=qt_sb, kt_pad=kt_pad, att_sb=att_sb,
                       v_sb=v_sb, outT=outT, small_out=small_out,
                       dram=dict(xt=xt_d, wq=wq_d, wk=wk_d, wv=wv_d,
                                 wo=wo_d, bq=bq_d, nrm=nrm_d))
            for rep in range(nrep):
                T = _emit_loads(ctx)
                _emit_A(ctx, T)
                if phases == "A":
                    _dbg_out(ctx, kt_pad[0])
                    break
                _emit_B(ctx, T)
                if phases == "AB":
                    _dbg_out(ctx, att_sb[0])
                    break
                _emit_C(ctx, T)
    nc.compile()
    return nc


class _Ctx:
    def __init__(self, **kw):
        self.__dict__.update(kw)


def _dbg_out(ctx, src):
    d = ctx.osp.tile([128, N // 2], ctx.f32, name="ost", tag="ost")
    ctx.nc.vector.tensor_copy(d[:, 0:512], src[:, 0:512])
    ctx.nc.sync.dma_start(out=ctx.outT[0:128, 0:512], in_=d[:, 0:512])


def _emit_loads(ctx):
    nc, fr, f32 = ctx.nc, ctx.fr, ctx.f32
    wp, d = ctx.wp, ctx.dram
    T = {}
    T["xt"] = wp.tile([128, KT_TILES * N], fr, name="xt", tag="xt")
    nc.gpsimd.dma_start(out=T["xt"][:], in_=d["xt"][:])
    T["wq"] = wp.tile([128, KT_TILES * CHC], fr, name="wq", tag="wq")
    nc.gpsimd.dma_start(out=T["wq"][:], in_=d["wq"][:])
    T["wk"] = wp.tile([128, KT_TILES * CHC], fr, name="wk", tag="wk")
    nc.gpsimd.dma_start(out=T["wk"][:], in_=d["wk"][:])
    T["wv"] = wp.tile([128, KT_TILES * CHC], fr, name="wv", tag="wv")
    nc.gpsimd.dma_start(out=T["wv"][:], in_=d["wv"][:])
    T["bq"] = wp.tile([128, 2], f32, name="bq", tag="bq")
    nc.gpsimd.dma_start(out=T["bq"][:], in_=d["bq"][:])
    # wo is read until the end of phase C; its WAR dep would block the
    # Pool queue (and everything behind it) until then — keep it on SP.
    T["wo"] = wp.tile([128, 2 * C], fr, name="wo", tag="wo")
    nc.sync.dma_start(out=T["wo"][:], in_=d["wo"][:])
    return T


def _emit_A(ctx, T):
    nc, ds, fr, f32 = ctx.nc, ctx.ds, ctx.fr, ctx.f32

    # QT / K: 4 query-chunk accumulators per weight slice so that
    # consecutive matmuls share the stationary operand
    for wmat in ("wq", "wk"):
        for t in range(2):
            accs = [ctx.ps.tile([128, 512], f32, name="acc", tag="p512",
                                bufs=4) for _ in range(4)]
            for k in range(KT_TILES):
                for qc in range(4):
                    nc.tensor.matmul(
                        accs[qc][:], T[wmat][:, ds(k * CHC + t * 128, 128)],
                        T["xt"][:, ds(k * N + qc * 512, 512)],
                        start=(k == 0), stop=(k == KT_TILES - 1))
            # drains alternate ACT/DVE so neither engine gates psum reuse
            for qc in range(4):
                if wmat == "wq":
                    if qc % 2 == 0:
                        nc.scalar.activation(
                            ctx.qt_sb[t][:, ds(qc * 512, 512)], accs[qc][:],
                            ctx.IDENT, bias=T["bq"][:, t:t + 1], scale=1.0)
                    else:
                        nc.vector.tensor_scalar_add(
                            ctx.qt_sb[t][:, ds(qc * 512, 512)], accs[qc][:],
                            T["bq"][:, t:t + 1])
                else:
                    # plain copies into the zero-padded per-head kt tiles
                    for i, h in enumerate((2 * t, 2 * t + 1)):
                        rows = slice(64 * i, 64 * i + 64)
                        if i == 0:
                            nc.scalar.activation(
                                ctx.kt_pad[h][rows, ds(qc * 512, 512)],
                                accs[qc][rows, :], ctx.IDENT, bias=0.0,
                                scale=1.0)
                        else:
                            nc.vector.tensor_copy(
                                ctx.kt_pad[h][rows, ds(qc * 512, 512)],
                                accs[qc][rows, :])

    # V in keys-major layout: V[keys, ch] = xT^T @ Wv (no bias: folded)
    for kt in range(RT_TILES):
        acc = ctx.ps.tile([128, 512], f32, name="vacc", tag="p512", bufs=4)
        for k in range(KT_TILES):
            nc.tensor.matmul(
                acc[:, 0:CHC], T["xt"][:, ds(k * N + kt * 128, 128)],
                T["wv"][:, ds(k * CHC, CHC)],
                start=(k == 0), stop=(k == KT_TILES - 1))
        for h in range(HPC):
            nc.vector.tensor_copy(
                ctx.v_sb[:, kt, ds(V_STORE[h], 64)], acc[:, ds(h * 64, 64)])


def _emit_B(ctx, T):
    nc, ds, fr, f32 = ctx.nc, ctx.ds, ctx.fr, ctx.f32
    qt_sb, kt_pad, att_sb, v_sb = (ctx.qt_sb, ctx.kt_pad, ctx.att_sb,
                                   ctx.v_sb)
    G = RT_TILES
    pending = [None]

    for h in range(HPC):
        tI, pO = h // 2, 64 * (h % 2)
        even = (h % 2 == 0)
        dr = 64 if even else 0   # denominator row in the av psum
        lb, lw = V_LHS[h]
        for qp in range(2):
            qA, qB = 2 * qp, 2 * qp + 1
            avs = []
            ets = [None] * G

            def emit_av(g, avs=avs, lb=lb, lw=lw, ets=ets):
                for j in range(2):   # consecutive MMs share the V panel
                    nc.tensor.matmul(
                        avs[j][0:lw, :], v_sb[:, g, ds(lb, lw)],
                        ets[g][:, ds(j * 512, 512)],
                        start=(g == 0), stop=(g == G - 1))

            for g in range(G):
                sc = ctx.ps.tile([128, 1024], f32, name="sc", tag="sc",
                                 bufs=2)
                for j, qc in enumerate((qA, qB)):
                    # full-128 contraction: kt_pad's zero rows null the
                    # other head's Q rows
                    nc.tensor.matmul(
                        sc[:, ds(j * 512, 512)],
                        kt_pad[h][:, ds(g * 128, 128)],
                        qt_sb[tI][:, ds(qc * 512, 512)],
                        start=True, stop=True)
                et = ctx.etp.tile([128, 1024], fr, name="et", tag="et")
                nc.scalar.activation(et[:], sc[:], ctx.EXPF, bias=0.0,
                                     scale=0.125)
                ets[g] = et
                if g == 1:
                    avs.extend(ctx.ps.tile([128, 512], f32, name="av",
                                           tag="p512", bufs=4)
                               for _ in range(2))
                if g == 4 and pending[0] is not None:
                    pending[0]()
                    pending[0] = None
                if g >= 2:         # AV runs at lag 2 behind the exp stream
                    emit_av(g - 2)
            emit_av(G - 2)
            emit_av(G - 1)

            # fast reciprocals of the two denominator rows, immediately
            # (DVE, ~51 ULP; denominators are sums of 2048 positive exps).
            # nrm row dr holds 1/denom; rows pO:pO+64 (disjoint) later hold
            # the PE-broadcast copy - one shared SBUF tile.
            nrm = ctx.bw.tile([128, 1024], f32, name="nrm", tag="nrm")
            for j in range(2):
                if dr == 0:
                    # fast approx recip (~51 ULP, ~5x) - but custom-DVE ops
                    # give wrong results at partition offsets > 0 on HW, so
                    # only the odd heads (denominator row 0) may use it
                    nc.vector.reciprocal_approx_fast(
                        out=nrm[dr:dr + 1, ds(j * 512, 512)],
                        in_=avs[j][dr:dr + 1, :])
                else:
                    # even heads: native reciprocal (offset-safe; 3.3us on
                    # DVE but off the PE critical path - the consuming
                    # broadcast is deferred 4 key tiles into the next block)
                    nc.vector.reciprocal(
                        nrm[dr:dr + 1, ds(j * 512, 512)],
                        avs[j][dr:dr + 1, :])

            def finish(avs=avs, nrm=nrm, tI=tI, pO=pO, dr=dr, qA=qA, qB=qB):
                # broadcast 1/denom across partitions via a DRAM round-trip
                # DMA (0-stride reads are legal from DRAM; zero PE cost)
                nc.sync.dma_start(out=ctx.dram["nrm"][0:1, :],
                                  in_=nrm[dr:dr + 1, :])
                nc.sync.dma_start(
                    out=nrm[pO:pO + 64, :],
                    in_=ctx.dram["nrm"][0:1, :].partition_broadcast(64))
                for j, qc in enumerate((qA, qB)):
                    nc.vector.tensor_mul(
                        att_sb[tI][pO:pO + 64, ds(qc * 512, 512)],
                        avs[j][pO:pO + 64, :],
                        nrm[pO:pO + 64, ds(j * 512, 512)])
            pending[0] = finish
    pending[0]()


def _emit_C(ctx, T):
    # 4 [128,512] accumulators per output row-block (p512 tag), drained
    # eagerly (plain copies - the output bias is added on the host while
    # unsharding) and DMA'd out in [128,1024] halves
    nc, ds, f32 = ctx.nc, ctx.ds, ctx.f32
    for m in range(8):
        for half in range(2):
            ost = ctx.osp.tile([128, N // 2], f32, name="ost", tag="ost")
            for j in range(2):
                qc = 2 * half + j
                acc = ctx.ps.tile([128, 512], f32, name="cacc", tag="p512",
                                  bufs=4)
                for t in range(2):
                    nc.tensor.matmul(
                        acc[:], T["wo"][:, ds(t * C + m * 128, 128)],
                        ctx.att_sb[t][:, ds(qc * 512, 512)],
                        start=(t == 0), stop=(t == 1))
                if j == 0:
                    nc.scalar.activation(ost[:, ds(j * 512, 512)], acc[:],
                                         ctx.IDENT, bias=0.0, scale=1.0)
                else:
                    nc.vector.tensor_copy(ost[:, ds(j * 512, 512)], acc[:])
            if ctx.small_out:
                if m == 0 and half == 0:
                    nc.sync.dma_start(out=ctx.outT[:, :], in_=ost[:, 0:512])
            else:
                nc.sync.dma_start(
                    out=ctx.outT[ds(m * 128, 128), ds(half * 1024, 1024)],
                    in_=ost[:])


# ---------------------------------------------------------------------------
# Host-side: runner (one-time jit) + kernel() entry point
# ---------------------------------------------------------------------------

class _SpmdRunner:
    def __init__(self, nc, n_cores=NCORES):
        import jax
        import numpy as _np
        from jax.sharding import Mesh, PartitionSpec
        from jax.experimental.shard_map import shard_map
        import concourse.mybir as mybir
        from concourse import bass2jax
        from concourse.bass2jax import _bass_exec_p, install_neuronx_cc_hook

        install_neuronx_cc_hook()
        self.jax = jax
        self.n_cores = n_cores
        partition_name = (nc.partition_id_tensor.name
                          if nc.partition_id_tensor else None)
        in_names, out_names, out_avals, zero_outs = [], [], [], []
        for alloc in nc.m.functions[0].allocations:
            if not isinstance(alloc, mybir.MemoryLocationSet):
                continue
            name = alloc.memorylocations[0].name
            if alloc.kind == "ExternalInput":
                if name != partition_name:
                    in_names.append(name)
            elif alloc.kind == "ExternalOutput":
                out_names.append(name)
                shape = tuple(alloc.tensor_shape)
                dtype = mybir.dt.np(alloc.dtype)
                out_avals.append(jax.core.ShapedArray(shape, dtype))
                zero_outs.append(_np.zeros(shape, dtype))
        self.in_names, self.out_names = in_names, out_names
        self.out_avals, self.zero_outs = out_avals, zero_outs
        n_params, n_outs = len(in_names), len(out_names)
        all_in = list(in_names) + list(out_names)
        if partition_name is not None:
            all_in.append(partition_name)
        donate = tuple(range(n_params, n_params + n_outs))

        def _body(*args):
            operands = list(args)
            if partition_name is not None:
                operands.append(bass2jax.partition_id_tensor())
            outs = _bass_exec_p.bind(
                *operands, out_avals=tuple(out_avals),
                in_names=tuple(all_in), out_names=tuple(out_names),
                lowering_input_output_aliases=(),
                sim_require_finite=True, sim_require_nnan=True, nc=nc)
            return tuple(outs)

        devices = jax.devices()[:n_cores]
        self.mesh = Mesh(_np.asarray(devices), ("core",))
        in_specs = (PartitionSpec("core"),) * (n_params + n_outs)
        out_specs = (PartitionSpec("core"),) * n_outs
        self.sharded = jax.jit(
            shard_map(_body, mesh=self.mesh, in_specs=in_specs,
                      out_specs=out_specs, check_rep=False),
            donate_argnums=donate, keep_unused=True)
        self._PartitionSpec = PartitionSpec

    def set_inputs(self, in_maps):
        import jax
        from jax.sharding import NamedSharding
        per_core = [[np.asarray(m[name]) for name in self.in_names]
                    for m in in_maps]
        sharding = NamedSharding(self.mesh, self._PartitionSpec("core"))
        self._in = [
            jax.device_put(np.concatenate(
                [per_core[c][i] for c in range(self.n_cores)], axis=0),
                sharding)
            for i in range(len(self.in_names))
        ]
        jax.block_until_ready(self._in)

    def run(self):
        import jax
        zeros = [np.zeros((self.n_cores * z.shape[0], *z.shape[1:]), z.dtype)
                 for z in self.zero_outs]
        out = self.sharded(*self._in, *zeros)
        jax.block_until_ready(out)
        return out

    def results(self, out_arrs):
        return [
            {name: np.asarray(out_arrs[i]).reshape(
                self.n_cores, *self.out_avals[i].shape)[c]
             for i, name in enumerate(self.out_names)}
            for c in range(self.n_cores)
        ]


def _get_runner(nrep: int = 1):
    key = ("runner", nrep, os.environ.get("MHA_KDTYPE", "f32r"))
    if key not in _CACHE:
        nc = _build_nc(nrep=nrep, kdtype=os.environ.get("MHA_KDTYPE", "f32r"))
        _CACHE[key] = _SpmdRunner(nc)
    return _CACHE[key]


def _make_in_maps(x, Wq, bq, Wk, bk, Wv, bv, Wo, bo):
    wq_f = np.asarray(Wq, np.float32)
    wk_f = np.asarray(Wk, np.float32)
    wv_f = np.asarray(Wv, np.float32)
    wo_f = np.asarray(Wo, np.float32)
    bq_f = np.asarray(bq, np.float32)
    bv_f = np.asarray(bv, np.float32)
    bo_f = np.asarray(bo, np.float32)
    x_f = np.asarray(x, np.float32)

    # xt host tiling: xt[p, k*N + n] = x[b][n, k*128+p]
    xts = []
    for b in range(B):
        xT = x_f[b].T                                  # [C, N]
        xts.append(np.ascontiguousarray(
            xT.reshape(KT_TILES, 128, N).transpose(1, 0, 2).reshape(128, -1)))

    in_maps = []
    for c in range(NCORES):
        b, hg = c // HPC, c % HPC
        ch = slice(CHC * hg, CHC * (hg + 1))
        # w[p, k*CHC + j] = W[k*128+p, ch0+j]
        wqc = np.ascontiguousarray(
            wq_f[:, ch].reshape(KT_TILES, 128, CHC).transpose(1, 0, 2)
            .reshape(128, -1))
        wkc = np.ascontiguousarray(
            wk_f[:, ch].reshape(KT_TILES, 128, CHC).transpose(1, 0, 2)
            .reshape(128, -1))
        wvc = np.ascontiguousarray(
            wv_f[:, ch].reshape(KT_TILES, 128, CHC).transpose(1, 0, 2)
            .reshape(128, -1))
        # wo[p, t*C + j] = Wo[ch0 + t*128 + p, j]
        woc = np.ascontiguousarray(
            wo_f[ch, :].reshape(2, 128, C).transpose(1, 0, 2).reshape(128, -1))
        bqc = np.ascontiguousarray(
            np.stack([bq_f[ch].reshape(2, 128)[0],
                      bq_f[ch].reshape(2, 128)[1]], axis=1))
        in_maps.append({"xt": xts[b], "wq": wqc, "wk": wkc, "wv": wvc,
                        "wo": woc, "bq": bqc})
    return in_maps


def kernel(x, Wq, bq, Wk, bk, Wv, bv, Wo, bo):
    runner = _get_runner()
    runner.set_inputs(_make_in_maps(x, Wq, bq, Wk, bk, Wv, bv, Wo, bo))
    res = runner.results(runner.run())
    out = np.zeros((B, N, C), np.float32)
    for c in range(NCORES):
        b = c // HPC
        out[b] += res[c]["outT"].T
    # output bias, plus the folded V bias (softmax weights sum to 1, so
    # attention(v + bv) = attention(v) + bv -> + bv @ Wo on the output)
    bias = (np.asarray(bo, np.float32)
            + np.asarray(bv, np.float32) @ np.asarray(Wo, np.float32))
    out += bias[None, None, :]
    return out


# revision 17
# speedup vs baseline: 1.7226x; 1.0363x over previous
"""MultiHeadAttention forward on 8 Trainium2 NeuronCores (Bass/Tile).

Problem: B=2, N=2048, C=1024, H=16, D=64, fp32.
  q/k/v = x @ W* + b*; scores = q k^T / sqrt(D); w = softmax(scores);
  out = (w v) @ Wo + bo.

Sharding: tensor-parallel over (batch, head-group). Core c handles batch
b = c//4 and heads 4*(c%4)..4*(c%4)+3 (channel slice of 256). Each core
computes its own Q/K/V projections, attention for its 4 heads, and a
PARTIAL output projection out_part = att @ Wo[ch, :]. The host sums the
4 partials per batch during unshard (row-parallel linear).

Bias identities exploited (exact in real arithmetic):
  - bk is DROPPED: scores[n,m] += q_n . bk is constant per query row n,
    and softmax over keys is invariant to per-row constants.
  - bv is FOLDED into the output bias: softmax weights sum to 1, so
    attention(v + bv) = attention(v) + bv; host bakes bv @ Wo_ch + bo/4
    into the per-core bo4 tile.

HW-calibrated notes (this machine, from perfetto traces):
  - f32r matmul streams 1 col/cycle @2.4GHz (213ns per 512-col matmul);
    LDWEIGHTS (~190-330ns) largely hides under the previous matmul's
    streaming. bf16/fp16 stream at the SAME rate - no dtype win.
  - ACT exp on [128,1024] is 1114ns flat regardless of dst dtype; the
    exp stream (128 tiles/rep) is the phase-B co-bottleneck with PE.
  - DVE ops are free-size-bound (~1.33ns/col); nc.vector.reciprocal is
    ~6.5ns/col (3.3us per [1,512] row!) - use reciprocal_approx_fast
    (~51 ULP, plenty for softmax denominators of O(100..3000)).
  - Every f32r matmul pays its weight load inline (standalone ldweights
    is broken for f32r); keep consecutive matmuls on the same stationary
    operand where convenient, but do not contort the schedule for it.

Per-core pipeline (all matmuls f32r, fp32 PSUM accumulation):
  A: QT[256,2048] = Wq_ch^T @ xT (+bq via ACT/DVE drains); K likewise
     into zero-padded per-head kt tiles (no bias); V[2048,256] in
     keys-major layout with per-head-pair panels [V_even |1| junk |
     V_odd] so each AV matmul also produces the softmax denominator row
     (ones column).
  B: per (head, query-block-pair): for each key tile g: scoresT
     [128,1024] (2 query blocks, shared stationary); et = exp(0.125 s)
     (ACT); AV matmuls run at lag 2 behind the exp stream (ets bufs=3)
     so the PE never waits on ACT latency. Normalization: fast DVE
     reciprocal of the denominator rows right after the last AV, then
     (deferred 2 key tiles into the next block) a PE ones-matmul
     broadcast + one DVE copy + DVE multiplies into att_sb.
  C: out_part^T [1024,2048] = Wo_ch^T @ attT, 4 [128,512] PSUM
     accumulators per output row-block, drained eagerly alternating
     ACT (bias via activation) / DVE (tensor_scalar_add), DMA out.

Host pre-tiles every input so each DMA is one large contiguous
transfer; loads go on the otherwise-idle GPSIMD queue so rep r+1's
transfers hide under rep r's phase B.
"""
import os
import numpy as np

B, N, C, H, D = 2, 2048, 1024, 16, 64
NCORES = 8
HPC = 4              # heads per core
CHC = HPC * D        # channels per core = 256
KT_TILES = 8         # contraction tiles over C
RT_TILES = 16        # key tiles over N

# v_sb per key-tile column layout: two pairs of 192 cols:
#   [V_h0(0:64) | ones(64) | junk(65:128) | V_h1(128:192)]
#   [V_h2(192:256) | ones(256) | junk(257:320) | V_h3(320:384)]
# (the junk region maps to av output rows nobody reads)
V_COLS = 384
V_STORE = (0, 128, 192, 320)   # where head h's V values are stored
# lhsT slices for the AV matmul: even heads [V|1] (65 cols, denom row 64);
# odd heads [1|junk|V] (128 cols, denom row 0, values rows 64..127)
V_LHS = ((0, 65), (64, 128), (192, 65), (256, 128))

_CACHE = {}


def _build_nc(nrep: int = 1, kdtype: str = "f32r", small_out: bool = False,
              phases: str = "ABC", vd_bf16: bool = False):
    import concourse.bacc as bacc
    import concourse.mybir as mybir
    import concourse.tile as tile
    from concourse.bass import ts, ds

    f32 = mybir.dt.float32
    fr = mybir.dt.float32r if kdtype == "f32r" else mybir.dt.float32

    nc = bacc.Bacc("TRN2", target_bir_lowering=False, debug=False)

    # ---- I/O (host pre-tiled; every load is one contiguous DMA) ----
    xt_d = nc.dram_tensor("xt", [128, KT_TILES * N], fr, kind="ExternalInput")
    wq_d = nc.dram_tensor("wq", [128, KT_TILES * CHC], fr, kind="ExternalInput")
    wk_d = nc.dram_tensor("wk", [128, KT_TILES * CHC], fr, kind="ExternalInput")
    wv_d = nc.dram_tensor("wv", [128, KT_TILES * CHC], fr, kind="ExternalInput")
    wo_d = nc.dram_tensor("wo", [128, 2 * C], fr, kind="ExternalInput")
    bq_d = nc.dram_tensor("bq", [128, 2], f32, kind="ExternalInput")
    nrm_d = nc.dram_tensor("nrm_scr", [1, 1024], f32, kind="Internal")
    outT = nc.dram_tensor("outT",
                          [128 if small_out else C, 512 if small_out else N],
                          f32, kind="ExternalOutput")

    EXPF = mybir.ActivationFunctionType.Exp

    with tile.TileContext(nc) as tc:
        # All pools live for the whole program; per-rep tiles rotate via
        # tags so cross-rep dependencies are per-tensor WAR (a per-rep pool
        # close would barrier rep r+1's loads on ALL of rep r).
        with tc.tile_pool(name="persist", bufs=1) as pp, \
             tc.tile_pool(name="wts", bufs=1) as wp, \
             tc.tile_pool(name="bwork", bufs=1) as bw, \
             tc.tile_pool(name="etp", bufs=3) as etp, \
             tc.tile_pool(name="ostp", bufs=1) as osp, \
             tc.tile_pool(name="ps", bufs=1, space="PSUM") as ps:
            qt_sb = [pp.tile([128, N], fr, name=f"qt{t}") for t in range(2)]
            # per-head zero-padded K: even h -> rows 0:64, odd h -> 64:128
            kt_pad = [pp.tile([128, N], fr, name=f"ktp{h}")
                      for h in range(HPC)]
            for h in range(HPC):
                z = slice(64, 128) if h % 2 == 0 else slice(0, 64)
                nc.vector.memset(kt_pad[h][z, :].bitcast(mybir.dt.uint32), 0)
            att_sb = [pp.tile([128, N], fr, name=f"att{t}") for t in range(2)]
            v_sb = pp.tile([128, RT_TILES, V_COLS], fr, name="v_sb")
            for base in (64, 256):   # ones + zero columns of the V panels
                nc.vector.memset(
                    v_sb[:, :, base].bitcast(mybir.dt.uint32), 0x3F800000)
                nc.vector.memset(
                    v_sb[:, :, base + 1:base + 64].bitcast(mybir.dt.uint32),
                    0)
            ctx = _Ctx(nc=nc, ds=ds, fr=fr, f32=f32, EXPF=EXPF,
                       IDENT=mybir.ActivationFunctionType.Identity,
                       wp=wp, bw=bw, etp=etp, osp=osp, ps=ps,
                       qt_sb=qt_sb, kt_pad=kt_pad, att_sb=att_sb,
                       v_sb=v_sb, outT=outT, small_out=small_out,
                       dram=dict(xt=xt_d, wq=wq_d, wk=wk_d, wv=wv_d,
                                 wo=wo_d, bq=bq_d, nrm=nrm_d))
            for rep in range(nrep):
                T = _emit_loads(ctx)
                _emit_A(ctx, T)
                if phases == "A":
                    _dbg_out(ctx, kt_pad[0])
                    break
                _emit_B(ctx, T)
                if phases == "AB":
                    _dbg_out(ctx, att_sb[0])
                    break
                _emit_C(ctx, T)
    nc.compile()
    return nc


class _Ctx:
    def __init__(self, **kw):
        self.__dict__.update(kw)


def _dbg_out(ctx, src):
    d = ctx.osp.tile([128, N // 2], ctx.f32, name="ost", tag="ost")
    ctx.nc.vector.tensor_copy(d[:, 0:512], src[:, 0:512])
    ctx.nc.sync.dma_start(out=ctx.outT[0:128, 0:512], in_=d[:, 0:512])


def _emit_loads(ctx):
    nc, fr, f32 = ctx.nc, ctx.fr, ctx.f32
    wp, d = ctx.wp, ctx.dram
    T = {}
    T["xt"] = wp.tile([128, KT_TILES * N], fr, name="xt", tag="xt")
    nc.gpsimd.dma_start(out=T["xt"][:], in_=d["xt"][:])
    T["wq"] = wp.tile([128, KT_TILES * CHC], fr, name="wq", tag="wq")
    nc.gpsimd.dma_start(out=T["wq"][:], in_=d["wq"][:])
    T["wk"] = wp.tile([128, KT_TILES * CHC], fr, name="wk", tag="wk")
    nc.gpsimd.dma_start(out=T["wk"][:], in_=d["wk"][:])
    T["wv"] = wp.tile([128, KT_TILES * CHC], fr, name="wv", tag="wv")
    nc.gpsimd.dma_start(out=T["wv"][:], in_=d["wv"][:])
    T["bq"] = wp.tile([128, 2], f32, name="bq", tag="bq")
    nc.gpsimd.dma_start(out=T["bq"][:], in_=d["bq"][:])
    # wo is read until the end of phase C; its WAR dep would block the
    # Pool queue (and everything behind it) until then — keep it on SP.
    T["wo"] = wp.tile([128, 2 * C], fr, name="wo", tag="wo")
    nc.sync.dma_start(out=T["wo"][:], in_=d["wo"][:])
    return T


def _emit_A(ctx, T):
    nc, ds, fr, f32 = ctx.nc, ctx.ds, ctx.fr, ctx.f32

    # QT / K: 4 query-chunk accumulators per weight slice so that
    # consecutive matmuls share the stationary operand
    for wmat in ("wq", "wk"):
        for t in range(2):
            accs = [ctx.ps.tile([128, 512], f32, name="acc",
                                tag=("p512" if i < 2 else "ia"), bufs=2)
                    for i in range(4)]
            for k in range(KT_TILES):
                for qc in range(4):
                    nc.tensor.matmul(
                        accs[qc][:], T[wmat][:, ds(k * CHC + t * 128, 128)],
                        T["xt"][:, ds(k * N + qc * 512, 512)],
                        start=(k == 0), stop=(k == KT_TILES - 1))
            # drains alternate ACT/DVE so neither engine gates psum reuse
            for qc in range(4):
                if wmat == "wq":
                    if qc % 2 == 0:
                        nc.scalar.activation(
                            ctx.qt_sb[t][:, ds(qc * 512, 512)], accs[qc][:],
                            ctx.IDENT, bias=T["bq"][:, t:t + 1], scale=1.0)
                    else:
                        nc.vector.tensor_scalar_add(
                            ctx.qt_sb[t][:, ds(qc * 512, 512)], accs[qc][:],
                            T["bq"][:, t:t + 1])
                else:
                    # plain copies into the zero-padded per-head kt tiles
                    for i, h in enumerate((2 * t, 2 * t + 1)):
                        rows = slice(64 * i, 64 * i + 64)
                        if i == 0:
                            nc.scalar.activation(
                                ctx.kt_pad[h][rows, ds(qc * 512, 512)],
                                accs[qc][rows, :], ctx.IDENT, bias=0.0,
                                scale=1.0)
                        else:
                            nc.vector.tensor_copy(
                                ctx.kt_pad[h][rows, ds(qc * 512, 512)],
                                accs[qc][rows, :])

    # V in keys-major layout: V[keys, ch] = xT^T @ Wv (no bias: folded)
    for kt in range(RT_TILES):
        acc = ctx.ps.tile([128, 512], f32, name="vacc", tag="p512", bufs=2)
        for k in range(KT_TILES):
            nc.tensor.matmul(
                acc[:, 0:CHC], T["xt"][:, ds(k * N + kt * 128, 128)],
                T["wv"][:, ds(k * CHC, CHC)],
                start=(k == 0), stop=(k == KT_TILES - 1))
        for h in range(HPC):
            nc.vector.tensor_copy(
                ctx.v_sb[:, kt, ds(V_STORE[h], 64)], acc[:, ds(h * 64, 64)])


def _emit_B(ctx, T):
    nc, ds, fr, f32 = ctx.nc, ctx.ds, ctx.fr, ctx.f32
    qt_sb, kt_pad, att_sb, v_sb = (ctx.qt_sb, ctx.kt_pad, ctx.att_sb,
                                   ctx.v_sb)
    G = RT_TILES

    for h in range(HPC):
        tI, pO = h // 2, 64 * (h % 2)
        even = (h % 2 == 0)
        dr = 64 if even else 0   # denominator row in the av psum
        lb, lw = V_LHS[h]
        for qp in range(2):
            qA, qB = 2 * qp, 2 * qp + 1
            avs = []
            ets = [None] * G

            def emit_av(g, avs=avs, lb=lb, lw=lw, ets=ets):
                for j in range(2):   # consecutive MMs share the V panel
                    nc.tensor.matmul(
                        avs[j][0:lw, :], v_sb[:, g, ds(lb, lw)],
                        ets[g][:, ds(j * 512, 512)],
                        start=(g == 0), stop=(g == G - 1))

            for g in range(G):
                sc = ctx.ps.tile([128, 1024], f32, name="sc", tag="sc",
                                 bufs=2)
                for j, qc in enumerate((qA, qB)):
                    # full-128 contraction: kt_pad's zero rows null the
                    # other head's Q rows
                    nc.tensor.matmul(
                        sc[:, ds(j * 512, 512)],
                        kt_pad[h][:, ds(g * 128, 128)],
                        qt_sb[tI][:, ds(qc * 512, 512)],
                        start=True, stop=True)
                et = ctx.etp.tile([128, 1024], fr, name="et", tag="et")
                nc.scalar.activation(et[:], sc[:], ctx.EXPF, bias=0.0,
                                     scale=0.125)
                ets[g] = et
                if g == 1:
                    avs.extend(ctx.ps.tile([128, 512], f32, name="av",
                                           tag="p512", bufs=2)
                               for _ in range(2))
                if g >= 2:         # AV runs at lag 2 behind the exp stream
                    emit_av(g - 2)
            emit_av(G - 2)
            emit_av(G - 1)

            # Drain the UNNORMALIZED av into att_sb right away (frees the
            # av psum bank for the next block), compute 1/denominators,
            # round-trip them through DRAM for the partition broadcast
            # (0-stride reads are legal from DRAM; zero PE cost), then
            # normalize att_sb in place. None of this touches the PE.
            # custom-DVE ops (the fast recip) are only correct at
            # partition offset 0 on HW, so: odd heads (denom row 0)
            # reciprocate THEN broadcast; even heads (denom row 64, values
            # rows 0:64) broadcast the RAW denominators then reciprocate
            # the [64,1024] broadcast at offset 0. Either way the slow
            # native reciprocal is avoided and nothing touches the PE.
            nrm = ctx.bw.tile([128, 1024], f32, name="nrm", tag="nrm")
            for j, qc in enumerate((qA, qB)):
                nc.vector.tensor_copy(
                    att_sb[tI][pO:pO + 64, ds(qc * 512, 512)],
                    avs[j][pO:pO + 64, :])
            for j in range(2):
                if dr == 0:
                    nc.vector.reciprocal_approx_fast(
                        out=nrm[dr:dr + 1, ds(j * 512, 512)],
                        in_=avs[j][dr:dr + 1, :])
                else:
                    nc.vector.tensor_copy(
                        nrm[dr:dr + 1, ds(j * 512, 512)],
                        avs[j][dr:dr + 1, :])
            nc.gpsimd.dma_start(out=ctx.dram["nrm"][0:1, :],
                                in_=nrm[dr:dr + 1, :])
            nc.gpsimd.dma_start(
                out=nrm[pO:pO + 64, :],
                in_=ctx.dram["nrm"][0:1, :].partition_broadcast(64))
            if dr != 0:
                nc.vector.reciprocal_approx_fast(
                    out=nrm[pO:pO + 64, :], in_=nrm[pO:pO + 64, :])
            for j, qc in enumerate((qA, qB)):
                nc.vector.tensor_mul(
                    att_sb[tI][pO:pO + 64, ds(qc * 512, 512)],
                    att_sb[tI][pO:pO + 64, ds(qc * 512, 512)],
                    nrm[pO:pO + 64, ds(j * 512, 512)])


def _emit_C(ctx, T):
    # t-outer so each wo slice stays stationary for 4 consecutive matmuls;
    # 4 [128,512] accumulators (p512 tag), drained eagerly after their t=1
    # matmul. Output bias is added on the host while unsharding.
    nc, ds, f32 = ctx.nc, ctx.ds, ctx.f32
    for m in range(8):
        osts = []
        accs = [ctx.ps.tile([128, 512], f32, name="cacc", tag="p512",
                            bufs=2) for _ in range(2)]
        accs += [ctx.ps.tile([128, 512], f32, name="cacc", tag="ia",
                             bufs=2) for _ in range(2)]
        for t in range(2):
            for qc in range(4):
                nc.tensor.matmul(
                    accs[qc][:], T["wo"][:, ds(t * C + m * 128, 128)],
                    ctx.att_sb[t][:, ds(qc * 512, 512)],
                    start=(t == 0), stop=(t == 1))
        for half in range(2):
            ost = ctx.osp.tile([128, N // 2], f32, name="ost", tag="ost")
            osts.append(ost)
            for j in range(2):
                qc = 2 * half + j
                if j == 0:
                    nc.scalar.activation(ost[:, ds(j * 512, 512)],
                                         accs[qc][:], ctx.IDENT, bias=0.0,
                                         scale=1.0)
                else:
                    nc.vector.tensor_copy(ost[:, ds(j * 512, 512)],
                                          accs[qc][:])
            if not ctx.small_out:
                nc.gpsimd.dma_start(
                    out=ctx.outT[ds(m * 128, 128), ds(half * 1024, 1024)],
                    in_=ost[:])
        if ctx.small_out and m == 0:
            nc.gpsimd.dma_start(out=ctx.outT[:, :], in_=osts[0][:, 0:512])


# ---------------------------------------------------------------------------
# Host-side: runner (one-time jit) + kernel() entry point
# ---------------------------------------------------------------------------

class _SpmdRunner:
    def __init__(self, nc, n_cores=NCORES):
        import jax
        import numpy as _np
        from jax.sharding import Mesh, PartitionSpec
        from jax.experimental.shard_map import shard_map
        import concourse.mybir as mybir
        from concourse import bass2jax
        from concourse.bass2jax import _bass_exec_p, install_neuronx_cc_hook

        install_neuronx_cc_hook()
        self.jax = jax
        self.n_cores = n_cores
        partition_name = (nc.partition_id_tensor.name
                          if nc.partition_id_tensor else None)
        in_names, out_names, out_avals, zero_outs = [], [], [], []
        for alloc in nc.m.functions[0].allocations:
            if not isinstance(alloc, mybir.MemoryLocationSet):
                continue
            name = alloc.memorylocations[0].name
            if alloc.kind == "ExternalInput":
                if name != partition_name:
                    in_names.append(name)
            elif alloc.kind == "ExternalOutput":
                out_names.append(name)
                shape = tuple(alloc.tensor_shape)
                dtype = mybir.dt.np(alloc.dtype)
                out_avals.append(jax.core.ShapedArray(shape, dtype))
                zero_outs.append(_np.zeros(shape, dtype))
        self.in_names, self.out_names = in_names, out_names
        self.out_avals, self.zero_outs = out_avals, zero_outs
        n_params, n_outs = len(in_names), len(out_names)
        all_in = list(in_names) + list(out_names)
        if partition_name is not None:
            all_in.append(partition_name)
        donate = tuple(range(n_params, n_params + n_outs))

        def _body(*args):
            operands = list(args)
            if partition_name is not None:
                operands.append(bass2jax.partition_id_tensor())
            outs = _bass_exec_p.bind(
                *operands, out_avals=tuple(out_avals),
                in_names=tuple(all_in), out_names=tuple(out_names),
                lowering_input_output_aliases=(),
                sim_require_finite=True, sim_require_nnan=True, nc=nc)
            return tuple(outs)

        devices = jax.devices()[:n_cores]
        self.mesh = Mesh(_np.asarray(devices), ("core",))
        in_specs = (PartitionSpec("core"),) * (n_params + n_outs)
        out_specs = (PartitionSpec("core"),) * n_outs
        self.sharded = jax.jit(
            shard_map(_body, mesh=self.mesh, in_specs=in_specs,
                      out_specs=out_specs, check_rep=False),
            donate_argnums=donate, keep_unused=True)
        self._PartitionSpec = PartitionSpec

    def set_inputs(self, in_maps):
        import jax
        from jax.sharding import NamedSharding
        per_core = [[np.asarray(m[name]) for name in self.in_names]
                    for m in in_maps]
        sharding = NamedSharding(self.mesh, self._PartitionSpec("core"))
        self._in = [
            jax.device_put(np.concatenate(
                [per_core[c][i] for c in range(self.n_cores)], axis=0),
                sharding)
            for i in range(len(self.in_names))
        ]
        jax.block_until_ready(self._in)

    def run(self):
        import jax
        zeros = [np.zeros((self.n_cores * z.shape[0], *z.shape[1:]), z.dtype)
                 for z in self.zero_outs]
        out = self.sharded(*self._in, *zeros)
        jax.block_until_ready(out)
        return out

    def results(self, out_arrs):
        return [
            {name: np.asarray(out_arrs[i]).reshape(
                self.n_cores, *self.out_avals[i].shape)[c]
             for i, name in enumerate(self.out_names)}
            for c in range(self.n_cores)
        ]


def _get_runner(nrep: int = 1):
    key = ("runner", nrep, os.environ.get("MHA_KDTYPE", "f32r"))
    if key not in _CACHE:
        nc = _build_nc(nrep=nrep, kdtype=os.environ.get("MHA_KDTYPE", "f32r"))
        _CACHE[key] = _SpmdRunner(nc)
    return _CACHE[key]


def _make_in_maps(x, Wq, bq, Wk, bk, Wv, bv, Wo, bo):
    wq_f = np.asarray(Wq, np.float32)
    wk_f = np.asarray(Wk, np.float32)
    wv_f = np.asarray(Wv, np.float32)
    wo_f = np.asarray(Wo, np.float32)
    bq_f = np.asarray(bq, np.float32)
    bv_f = np.asarray(bv, np.float32)
    bo_f = np.asarray(bo, np.float32)
    x_f = np.asarray(x, np.float32)

    # xt host tiling: xt[p, k*N + n] = x[b][n, k*128+p]
    xts = []
    for b in range(B):
        xT = x_f[b].T                                  # [C, N]
        xts.append(np.ascontiguousarray(
            xT.reshape(KT_TILES, 128, N).transpose(1, 0, 2).reshape(128, -1)))

    in_maps = []
    for c in range(NCORES):
        b, hg = c // HPC, c % HPC
        ch = slice(CHC * hg, CHC * (hg + 1))
        # w[p, k*CHC + j] = W[k*128+p, ch0+j]
        wqc = np.ascontiguousarray(
            wq_f[:, ch].reshape(KT_TILES, 128, CHC).transpose(1, 0, 2)
            .reshape(128, -1))
        wkc = np.ascontiguousarray(
            wk_f[:, ch].reshape(KT_TILES, 128, CHC).transpose(1, 0, 2)
            .reshape(128, -1))
        wvc = np.ascontiguousarray(
            wv_f[:, ch].reshape(KT_TILES, 128, CHC).transpose(1, 0, 2)
            .reshape(128, -1))
        # wo[p, t*C + j] = Wo[ch0 + t*128 + p, j]
        woc = np.ascontiguousarray(
            wo_f[ch, :].reshape(2, 128, C).transpose(1, 0, 2).reshape(128, -1))
        bqc = np.ascontiguousarray(
            np.stack([bq_f[ch].reshape(2, 128)[0],
                      bq_f[ch].reshape(2, 128)[1]], axis=1))
        in_maps.append({"xt": xts[b], "wq": wqc, "wk": wkc, "wv": wvc,
                        "wo": woc, "bq": bqc})
    return in_maps


def kernel(x, Wq, bq, Wk, bk, Wv, bv, Wo, bo):
    runner = _get_runner()
    runner.set_inputs(_make_in_maps(x, Wq, bq, Wk, bk, Wv, bv, Wo, bo))
    res = runner.results(runner.run())
    out = np.zeros((B, N, C), np.float32)
    for c in range(NCORES):
        b = c // HPC
        out[b] += res[c]["outT"].T
    # output bias, plus the folded V bias (softmax weights sum to 1, so
    # attention(v + bv) = attention(v) + bv -> + bv @ Wo on the output)
    bias = (np.asarray(bo, np.float32)
            + np.asarray(bv, np.float32) @ np.asarray(Wo, np.float32))
    out += bias[None, None, :]
    return out
